# revision 6
# baseline (speedup 1.0000x reference)
"""Trainium2 Bass kernel for sigmoid multi-head attention (B=4, N=2048, C=1024, H=16).

Strategy: 8 cores = 4 batches x 2 head-groups (8 heads each). Each core is fully
independent (no collectives):
  - Host pre-transposes + pre-casts inputs to bf16: x^T [C,N], W^T slices.
  - Device computes q^T,k^T (transposed) and v (natural) projections, then per head:
    scores^T[nk,nq] = k^T_h.T @ q^T_h  -> sigmoid (scaled) -> attn^T bf16
    (written to DRAM in [h, nk, nq] layout; host un-transposes),
    out^T_h[d,nq] accumulated as v_h.T @ attn^T, then partial projection
    out_part[nq,C] = outz^T.T @ Wp^T-slice.
  - Host: out[b] = part(core0) + part(core1) + bp; attn un-transposed per head.
"""

import numpy as np
import ml_dtypes

B, N, C, H = 4, 2048, 1024, 16
D = C // H            # 64
HPC = H // 2          # 8 heads per core
CL = HPC * D          # 512 local channels
NCORES = 8
SCALE = D ** -0.5

P = 128
KT = C // P           # 8  k-tiles over c_in
MT = CL // P          # 4  tiles over local channels
NT = N // P           # 16 tiles over sequence
NB = N // 512         # 4  512-wide banks over sequence

_BF16 = ml_dtypes.bfloat16

_CACHED_NC = None


def _build():
    import concourse.mybir as mybir
    import concourse.tile as tile
    from concourse import bacc

    bf16 = mybir.dt.bfloat16
    f32 = mybir.dt.float32
    SIG = mybir.ActivationFunctionType.Sigmoid

    nc = bacc.Bacc("TRN2")

    xqT = nc.declare_dram_parameter("xqT", [C, N], bf16, isOutput=False)
    xkT = nc.declare_dram_parameter("xkT", [C, N], bf16, isOutput=False)
    xvT = nc.declare_dram_parameter("xvT", [C, N], bf16, isOutput=False)
    wqT = nc.declare_dram_parameter("wqT", [C, CL], bf16, isOutput=False)
    wkT = nc.declare_dram_parameter("wkT", [C, CL], bf16, isOutput=False)
    wvT = nc.declare_dram_parameter("wvT", [C, CL], bf16, isOutput=False)
    wpT = nc.declare_dram_parameter("wpT", [CL, C], bf16, isOutput=False)
    attn_out = nc.declare_dram_parameter("attn_out", [HPC, N, N], bf16, isOutput=True)
    out_part = nc.declare_dram_parameter("out_part", [N, C], bf16, isOutput=True)

    with tile.TileContext(nc) as tc:
        with (
            tc.tile_pool(name="big", bufs=16) as pool_big,     # x k-tiles + attnT strips
            tc.tile_pool(name="qk", bufs=2 * MT) as pool_qk,   # qT + kT tiles, live all run
            tc.tile_pool(name="vp", bufs=NT) as pool_v,        # v tiles, live all run
            tc.tile_pool(name="w", bufs=3 * KT) as pool_w,     # wq/wk/wv k-tiles
            tc.tile_pool(name="wp", bufs=MT) as pool_wp,
            tc.tile_pool(name="oz", bufs=MT) as pool_oz,       # outz^T bf16
            tc.tile_pool(name="ob", bufs=4) as pool_ob,        # final out staging
            tc.tile_pool(name="psA", bufs=4, space="PSUM") as pool_psA,  # [128,512] 1-bank tiles
            tc.tile_pool(name="psS", bufs=2, space="PSUM") as pool_psS,  # [128,1024] scores tiles
        ):
            # ---- weight loads ----
            w_tiles = {}
            for name, dram in (("q", wqT), ("k", wkT), ("v", wvT)):
                for kt in range(KT):
                    t = pool_w.tile([P, CL], bf16, tag="w")
                    nc.sync.dma_start(out=t, in_=dram[kt * P:(kt + 1) * P, :])
                    w_tiles[(name, kt)] = t
            wp_tiles = []
            for kt in range(MT):
                t = pool_wp.tile([P, C], bf16, tag="wp")
                nc.sync.dma_start(out=t, in_=wpT[kt * P:(kt + 1) * P, :])
                wp_tiles.append(t)

            # ---- projections (x loads interleaved per tensor to bound SBUF) ----
            qT_tiles = []   # [128, N] x MT  (c_out_local on partitions)
            kT_tiles = []
            v_tiles = []    # [128, CL] x NT (n on partitions)

            for name, dram, outlist in (("q", xqT, qT_tiles), ("k", xkT, kT_tiles)):
                x_tiles = []
                for kt in range(KT):
                    t = pool_big.tile([P, N], bf16, tag="big")
                    nc.sync.dma_start(out=t, in_=dram[kt * P:(kt + 1) * P, :])
                    x_tiles.append(t)
                for mi in range(MT):
                    out_t = pool_qk.tile([P, N], bf16, tag="qk")
                    for nb in range(NB):
                        ps = pool_psA.tile([P, 512], f32, tag="ps")
                        for kt in range(KT):
                            nc.tensor.matmul(
                                ps,
                                lhsT=w_tiles[(name, kt)][:, mi * P:(mi + 1) * P],
                                rhs=x_tiles[kt][:, nb * 512:(nb + 1) * 512],
                                start=(kt == 0),
                                stop=(kt == KT - 1),
                            )
                        nc.vector.tensor_copy(
                            out=out_t[:, nb * 512:(nb + 1) * 512], in_=ps
                        )
                    outlist.append(out_t)

            xv_tiles = []
            for kt in range(KT):
                t = pool_big.tile([P, N], bf16, tag="big")
                nc.sync.dma_start(out=t, in_=xvT[kt * P:(kt + 1) * P, :])
                xv_tiles.append(t)
            for ni in range(NT):
                out_t = pool_v.tile([P, CL], bf16, tag="v")
                ps = pool_psA.tile([P, 512], f32, tag="ps")
                for kt in range(KT):
                    nc.tensor.matmul(
                        ps,
                        lhsT=xv_tiles[kt][:, ni * P:(ni + 1) * P],
                        rhs=w_tiles[("v", kt)],
                        start=(kt == 0),
                        stop=(kt == KT - 1),
                    )
                nc.vector.tensor_copy(out=out_t, in_=ps)
                v_tiles.append(out_t)

            # ---- attention, one head at a time ----
            outz_tiles = [
                pool_oz.tile([P, N], bf16, tag="oz", name=f"outz_{mi}")
                for mi in range(MT)
            ]

            # Process heads in pairs: scores of the two heads use disjoint PE row
            # groups (base partition 0 / 64) and run concurrently; attn@v packs
            # the two heads into disjoint PE column groups via tile_position,
            # accumulating into disjoint partition halves of shared PSUM banks.
            for hp in range(HPC // 2):
                avps = [
                    pool_psA.tile([P, 512], f32, tag="ps", name=f"avps_{hp}_{qb}")
                    for qb in range(NB)
                ]
                for ni in range(NT):
                    strips = []
                    for sub in range(2):
                        hl = hp * 2 + sub
                        po = sub * D
                        strip = pool_big.tile([P, N], bf16, tag="big", name=f"strip{sub}")
                        strips.append(strip)
                        for qh in range(2):  # two 1024-wide sigmoid chunks
                            sps = pool_psS.tile([P, 1024], f32, tag="sps", name="sps")
                            for qq in range(2):
                                qb = qh * 2 + qq
                                nc.tensor.matmul(
                                    sps[:, qq * 512:(qq + 1) * 512],
                                    lhsT=kT_tiles[hp][po:po + D, ni * P:(ni + 1) * P],
                                    rhs=qT_tiles[hp][po:po + D, qb * 512:(qb + 1) * 512],
                                    start=True,
                                    stop=True,
                                )
                            nc.scalar.activation(
                                strip[:, qh * 1024:(qh + 1) * 1024], sps, SIG,
                                scale=SCALE,
                            )
                        nc.sync.dma_start(
                            out=attn_out[hl, ni * P:(ni + 1) * P, :], in_=strip
                        )
                    for qb in range(NB):
                        for sub in range(2):
                            hl = hp * 2 + sub
                            nc.tensor.matmul(
                                avps[qb][sub * D:(sub + 1) * D, :],
                                lhsT=v_tiles[ni][:, hl * D:(hl + 1) * D],
                                rhs=strips[sub][:, qb * 512:(qb + 1) * 512],
                                start=(ni == 0),
                                stop=(ni == NT - 1),
                                tile_position=(0, sub * D),
                            )
                for qb in range(NB):
                    nc.vector.tensor_copy(
                        out=outz_tiles[hp][:, qb * 512:(qb + 1) * 512],
                        in_=avps[qb],
                    )

            # ---- final projection: out_part[nq, C] = outz^T.T @ wpT ----
            for mo in range(NT):
                ob = pool_ob.tile([P, C], bf16, tag="ob")
                for cb in range(2):
                    ps = pool_psA.tile([P, 512], f32, tag="ps")
                    for kt in range(MT):
                        nc.tensor.matmul(
                            ps,
                            lhsT=outz_tiles[kt][:, mo * P:(mo + 1) * P],
                            rhs=wp_tiles[kt][:, cb * 512:(cb + 1) * 512],
                            start=(kt == 0),
                            stop=(kt == MT - 1),
                        )
                    nc.vector.tensor_copy(out=ob[:, cb * 512:(cb + 1) * 512], in_=ps)
                nc.sync.dma_start(out=out_part[mo * P:(mo + 1) * P, :], in_=ob)

    nc.compile()
    return nc


def _get_nc():
    global _CACHED_NC
    if _CACHED_NC is None:
        _CACHED_NC = _build()
    return _CACHED_NC


def _make_in_maps(inputs):
    in_maps = []
    for i in range(NCORES):
        b, g = i // 2, i % 2
        lo, hi = g * CL, (g + 1) * CL
        in_maps.append({
            "xqT": np.asarray(inputs["x_q"][b]).T.astype(_BF16),
            "xkT": np.asarray(inputs["x_k"][b]).T.astype(_BF16),
            "xvT": np.asarray(inputs["x_v"][b]).T.astype(_BF16),
            "wqT": np.asarray(inputs["Wq"])[lo:hi, :].T.astype(_BF16),
            "wkT": np.asarray(inputs["Wk"])[lo:hi, :].T.astype(_BF16),
            "wvT": np.asarray(inputs["Wv"])[lo:hi, :].T.astype(_BF16),
            "wpT": np.asarray(inputs["Wp"])[:, lo:hi].T.astype(_BF16),
        })
    return in_maps


def _assemble(results, inputs):
    out = np.zeros((B, N, C), np.float32)
    attn = np.empty((B, H, N, N), np.float32)
    for i in range(NCORES):
        b, g = i // 2, i % 2
        r = results[i]
        attn[b, g * HPC:(g + 1) * HPC] = (
            np.asarray(r["attn_out"]).astype(np.float32).transpose(0, 2, 1)
        )
        out[b] += np.asarray(r["out_part"]).astype(np.float32)
    out += np.asarray(inputs["bp"]).astype(np.float32)[None, None, :]
    return out, attn


def run(inputs, trace=False, **kwargs):
    from concourse.bass_utils import run_bass_kernel_spmd

    nc = _get_nc()
    in_maps = _make_in_maps(inputs)
    res = run_bass_kernel_spmd(
        nc, in_maps, core_ids=list(range(NCORES)), trace=trace, **kwargs
    )
    out, attn = _assemble(res.results, inputs)
    return (out, attn), res


def kernel(**inputs):
    (out, attn), _ = run(inputs)
    return out, attn


# revision 7
# speedup vs baseline: 1.0028x; 1.0028x over previous
"""Trainium2 Bass kernel for sigmoid multi-head attention (B=4, N=2048, C=1024, H=16).

Strategy: 8 cores = 4 batches x 2 head-groups (8 heads each). Each core is fully
independent (no collectives):
  - Host pre-transposes + pre-casts inputs to bf16: x^T [C,N], W^T slices.
  - Device computes q^T,k^T (transposed) and v (natural) projections, then per head:
    scores^T[nk,nq] = k^T_h.T @ q^T_h  -> sigmoid (scaled) -> attn^T bf16
    (written to DRAM in [h, nk, nq] layout; host un-transposes),
    out^T_h[d,nq] accumulated as v_h.T @ attn^T, then partial projection
    out_part[nq,C] = outz^T.T @ Wp^T-slice.
  - Host: out[b] = part(core0) + part(core1) + bp; attn un-transposed per head.
"""

import numpy as np
import ml_dtypes

B, N, C, H = 4, 2048, 1024, 16
D = C // H            # 64
HPC = H // 2          # 8 heads per core
CL = HPC * D          # 512 local channels
NCORES = 8
SCALE = D ** -0.5

P = 128
KT = C // P           # 8  k-tiles over c_in
MT = CL // P          # 4  tiles over local channels
NT = N // P           # 16 tiles over sequence
NB = N // 512         # 4  512-wide banks over sequence

_BF16 = ml_dtypes.bfloat16

_CACHED_NC = None


def _build():
    import concourse.mybir as mybir
    import concourse.tile as tile
    from concourse import bacc

    bf16 = mybir.dt.bfloat16
    f32 = mybir.dt.float32
    SIG = mybir.ActivationFunctionType.Sigmoid

    nc = bacc.Bacc("TRN2")

    xqT = nc.declare_dram_parameter("xqT", [C, N], bf16, isOutput=False)
    xkT = nc.declare_dram_parameter("xkT", [C, N], bf16, isOutput=False)
    xvT = nc.declare_dram_parameter("xvT", [C, N], bf16, isOutput=False)
    wqT = nc.declare_dram_parameter("wqT", [C, CL], bf16, isOutput=False)
    wkT = nc.declare_dram_parameter("wkT", [C, CL], bf16, isOutput=False)
    wvT = nc.declare_dram_parameter("wvT", [C, CL], bf16, isOutput=False)
    wpT = nc.declare_dram_parameter("wpT", [CL, C], bf16, isOutput=False)
    attn_out = nc.declare_dram_parameter("attn_out", [HPC, N, N], bf16, isOutput=True)
    out_part = nc.declare_dram_parameter("out_part", [N, C], bf16, isOutput=True)

    with tile.TileContext(nc) as tc:
        with (
            tc.tile_pool(name="big", bufs=16) as pool_big,     # x k-tiles + attnT strips
            tc.tile_pool(name="qk", bufs=2 * MT) as pool_qk,   # qT + kT tiles, live all run
            tc.tile_pool(name="vp", bufs=NT) as pool_v,        # v tiles, live all run
            tc.tile_pool(name="w", bufs=3 * KT) as pool_w,     # wq/wk/wv k-tiles
            tc.tile_pool(name="wp", bufs=MT) as pool_wp,
            tc.tile_pool(name="oz", bufs=MT) as pool_oz,       # outz^T bf16
            tc.tile_pool(name="ob", bufs=4) as pool_ob,        # final out staging
            tc.tile_pool(name="psA", bufs=4, space="PSUM") as pool_psA,  # [128,512] 1-bank tiles
            tc.tile_pool(name="psS", bufs=2, space="PSUM") as pool_psS,  # [128,1024] scores tiles
        ):
            # ---- weight loads ----
            w_tiles = {}
            for name, dram in (("q", wqT), ("k", wkT), ("v", wvT)):
                for kt in range(KT):
                    t = pool_w.tile([P, CL], bf16, tag="w")
                    nc.sync.dma_start(out=t, in_=dram[kt * P:(kt + 1) * P, :])
                    w_tiles[(name, kt)] = t
            wp_tiles = []
            for kt in range(MT):
                t = pool_wp.tile([P, C], bf16, tag="wp")
                nc.sync.dma_start(out=t, in_=wpT[kt * P:(kt + 1) * P, :])
                wp_tiles.append(t)

            # ---- projections (x loads interleaved per tensor to bound SBUF) ----
            qT_tiles = []   # [128, N] x MT  (c_out_local on partitions)
            kT_tiles = []
            v_tiles = []    # [128, CL] x NT (n on partitions)

            for name, dram, outlist in (("q", xqT, qT_tiles), ("k", xkT, kT_tiles)):
                x_tiles = []
                for kt in range(KT):
                    t = pool_big.tile([P, N], bf16, tag="big")
                    nc.sync.dma_start(out=t, in_=dram[kt * P:(kt + 1) * P, :])
                    x_tiles.append(t)
                for mi in range(MT):
                    out_t = pool_qk.tile([P, N], bf16, tag="qk")
                    for nb in range(NB):
                        ps = pool_psA.tile([P, 512], f32, tag="ps")
                        for kt in range(KT):
                            nc.tensor.matmul(
                                ps,
                                lhsT=w_tiles[(name, kt)][:, mi * P:(mi + 1) * P],
                                rhs=x_tiles[kt][:, nb * 512:(nb + 1) * 512],
                                start=(kt == 0),
                                stop=(kt == KT - 1),
                            )
                        nc.vector.tensor_copy(
                            out=out_t[:, nb * 512:(nb + 1) * 512], in_=ps
                        )
                    outlist.append(out_t)

            xv_tiles = []
            for kt in range(KT):
                t = pool_big.tile([P, N], bf16, tag="big")
                nc.sync.dma_start(out=t, in_=xvT[kt * P:(kt + 1) * P, :])
                xv_tiles.append(t)
            for ni in range(NT):
                out_t = pool_v.tile([P, CL], bf16, tag="v")
                ps = pool_psA.tile([P, 512], f32, tag="ps")
                for kt in range(KT):
                    nc.tensor.matmul(
                        ps,
                        lhsT=xv_tiles[kt][:, ni * P:(ni + 1) * P],
                        rhs=w_tiles[("v", kt)],
                        start=(kt == 0),
                        stop=(kt == KT - 1),
                    )
                nc.vector.tensor_copy(out=out_t, in_=ps)
                v_tiles.append(out_t)

            # ---- attention, one head at a time ----
            outz_tiles = [
                pool_oz.tile([P, N], bf16, tag="oz", name=f"outz_{mi}")
                for mi in range(MT)
            ]

            # Process heads in pairs: scores of the two heads use disjoint PE row
            # groups (base partition 0 / 64) and run concurrently; attn@v packs
            # the two heads into disjoint PE column groups via tile_position,
            # accumulating into disjoint partition halves of shared PSUM banks.
            for hp in range(HPC // 2):
                avps = [
                    pool_psA.tile([P, 512], f32, tag="ps", name=f"avps_{hp}_{qb}")
                    for qb in range(NB)
                ]
                for ni in range(NT):
                    # scores for both heads, emission alternating PE row groups
                    # (sub 0 -> rows 0-63, sub 1 -> rows 64-127) so consecutive
                    # matmuls execute concurrently on disjoint sub-arrays.
                    strips = [
                        pool_big.tile([P, N], bf16, tag="big", name=f"strip{s}")
                        for s in range(2)
                    ]
                    for qh in range(2):  # two 1024-wide sigmoid chunks
                        spss = [
                            pool_psS.tile([P, 1024], f32, tag="sps", name=f"sps{s}")
                            for s in range(2)
                        ]
                        for qq in range(2):
                            qb = qh * 2 + qq
                            for sub in range(2):
                                po = sub * D
                                nc.tensor.matmul(
                                    spss[sub][:, qq * 512:(qq + 1) * 512],
                                    lhsT=kT_tiles[hp][po:po + D, ni * P:(ni + 1) * P],
                                    rhs=qT_tiles[hp][po:po + D, qb * 512:(qb + 1) * 512],
                                    start=True,
                                    stop=True,
                                )
                        for sub in range(2):
                            nc.scalar.activation(
                                strips[sub][:, qh * 1024:(qh + 1) * 1024], spss[sub],
                                SIG, scale=SCALE,
                            )
                    for sub in range(2):
                        nc.sync.dma_start(
                            out=attn_out[hp * 2 + sub, ni * P:(ni + 1) * P, :],
                            in_=strips[sub],
                        )
                    for qb in range(NB):
                        for sub in range(2):
                            hl = hp * 2 + sub
                            nc.tensor.matmul(
                                avps[qb][sub * D:(sub + 1) * D, :],
                                lhsT=v_tiles[ni][:, hl * D:(hl + 1) * D],
                                rhs=strips[sub][:, qb * 512:(qb + 1) * 512],
                                start=(ni == 0),
                                stop=(ni == NT - 1),
                                tile_position=(0, sub * D),
                            )
                for qb in range(NB):
                    nc.vector.tensor_copy(
                        out=outz_tiles[hp][:, qb * 512:(qb + 1) * 512],
                        in_=avps[qb],
                    )

            # ---- final projection: out_part[nq, C] = outz^T.T @ wpT ----
            for mo in range(NT):
                ob = pool_ob.tile([P, C], bf16, tag="ob")
                for cb in range(2):
                    ps = pool_psA.tile([P, 512], f32, tag="ps")
                    for kt in range(MT):
                        nc.tensor.matmul(
                            ps,
                            lhsT=outz_tiles[kt][:, mo * P:(mo + 1) * P],
                            rhs=wp_tiles[kt][:, cb * 512:(cb + 1) * 512],
                            start=(kt == 0),
                            stop=(kt == MT - 1),
                        )
                    nc.vector.tensor_copy(out=ob[:, cb * 512:(cb + 1) * 512], in_=ps)
                nc.sync.dma_start(out=out_part[mo * P:(mo + 1) * P, :], in_=ob)

    nc.compile()
    return nc


def _get_nc():
    global _CACHED_NC
    if _CACHED_NC is None:
        _CACHED_NC = _build()
    return _CACHED_NC


def _make_in_maps(inputs):
    in_maps = []
    for i in range(NCORES):
        b, g = i // 2, i % 2
        lo, hi = g * CL, (g + 1) * CL
        in_maps.append({
            "xqT": np.asarray(inputs["x_q"][b]).T.astype(_BF16),
            "xkT": np.asarray(inputs["x_k"][b]).T.astype(_BF16),
            "xvT": np.asarray(inputs["x_v"][b]).T.astype(_BF16),
            "wqT": np.asarray(inputs["Wq"])[lo:hi, :].T.astype(_BF16),
            "wkT": np.asarray(inputs["Wk"])[lo:hi, :].T.astype(_BF16),
            "wvT": np.asarray(inputs["Wv"])[lo:hi, :].T.astype(_BF16),
            "wpT": np.asarray(inputs["Wp"])[:, lo:hi].T.astype(_BF16),
        })
    return in_maps


def _assemble(results, inputs):
    out = np.zeros((B, N, C), np.float32)
    attn = np.empty((B, H, N, N), np.float32)
    for i in range(NCORES):
        b, g = i // 2, i % 2
        r = results[i]
        attn[b, g * HPC:(g + 1) * HPC] = (
            np.asarray(r["attn_out"]).astype(np.float32).transpose(0, 2, 1)
        )
        out[b] += np.asarray(r["out_part"]).astype(np.float32)
    out += np.asarray(inputs["bp"]).astype(np.float32)[None, None, :]
    return out, attn


def run(inputs, trace=False, **kwargs):
    from concourse.bass_utils import run_bass_kernel_spmd

    nc = _get_nc()
    in_maps = _make_in_maps(inputs)
    res = run_bass_kernel_spmd(
        nc, in_maps, core_ids=list(range(NCORES)), trace=trace, **kwargs
    )
    out, attn = _assemble(res.results, inputs)
    return (out, attn), res


def kernel(**inputs):
    (out, attn), _ = run(inputs)
    return out, attn


# revision 8
# speedup vs baseline: 1.0261x; 1.0232x over previous
"""Trainium2 Bass kernel for sigmoid multi-head attention (B=4, N=2048, C=1024, H=16).

Strategy: 8 cores = 4 batches x 2 head-groups (8 heads each). Each core is fully
independent (no collectives):
  - Host pre-transposes + pre-casts inputs to bf16: x^T [C,N], W^T slices.
  - Device computes q^T,k^T (transposed) and v (natural) projections, then per head:
    scores^T[nk,nq] = k^T_h.T @ q^T_h  -> sigmoid (scaled) -> attn^T bf16
    (written to DRAM in [h, nk, nq] layout; host un-transposes),
    out^T_h[d,nq] accumulated as v_h.T @ attn^T, then partial projection
    out_part[nq,C] = outz^T.T @ Wp^T-slice.
  - Host: out[b] = part(core0) + part(core1) + bp; attn un-transposed per head.

All attention matmuls are zero-padded to full 128x128 PE tiles: k^T is stored
per head with the other head's partition rows zeroed, and v is stored per head
parity with the other parity's columns zeroed, so head-pair outputs stack /
accumulate through the zeros. Full-tile matmuls keep LDWEIGHTS on the
background buffer path (~225 ns/MM) instead of serializing (~270-330 ns/MM).
"""

import numpy as np
import ml_dtypes

B, N, C, H = 4, 2048, 1024, 16
D = C // H            # 64
HPC = H // 2          # 8 heads per core
CL = HPC * D          # 512 local channels
NCORES = 8
SCALE = D ** -0.5

P = 128
KT = C // P           # 8  k-tiles over c_in
MT = CL // P          # 4  tiles over local channels
NT = N // P           # 16 tiles over sequence
NB = N // 512         # 4  512-wide banks over sequence

_BF16 = ml_dtypes.bfloat16

_CACHED_NC = None


def _build():
    import concourse.mybir as mybir
    import concourse.tile as tile
    from concourse import bacc

    bf16 = mybir.dt.bfloat16
    f32 = mybir.dt.float32
    SIG = mybir.ActivationFunctionType.Sigmoid

    nc = bacc.Bacc("TRN2")

    xqT = nc.declare_dram_parameter("xqT", [C, N], bf16, isOutput=False)
    xkT = nc.declare_dram_parameter("xkT", [C, N], bf16, isOutput=False)
    xvT = nc.declare_dram_parameter("xvT", [C, N], bf16, isOutput=False)
    wqT = nc.declare_dram_parameter("wqT", [C, CL], bf16, isOutput=False)
    wkT = nc.declare_dram_parameter("wkT", [C, CL], bf16, isOutput=False)
    wvT = nc.declare_dram_parameter("wvT", [C, CL], bf16, isOutput=False)
    wpT = nc.declare_dram_parameter("wpT", [CL, C], bf16, isOutput=False)
    attn_out = nc.declare_dram_parameter("attn_out", [HPC, N, N], bf16, isOutput=True)
    out_part = nc.declare_dram_parameter("out_part", [N, C], bf16, isOutput=True)

    with tile.TileContext(nc) as tc:
        with (
            tc.tile_pool(name="big", bufs=14) as pool_big,     # x k-tiles + attnT strips
            tc.tile_pool(name="qt", bufs=MT) as pool_qt,       # qT tiles, live all run
            tc.tile_pool(name="kp", bufs=HPC) as pool_kp,      # padded kT per head
            tc.tile_pool(name="vp", bufs=2 * NT) as pool_v,    # padded v per parity
            tc.tile_pool(name="w", bufs=3 * KT) as pool_w,     # wq/wk/wv k-tiles
            tc.tile_pool(name="wp", bufs=MT) as pool_wp,
            tc.tile_pool(name="oz", bufs=MT) as pool_oz,       # outz^T bf16
            tc.tile_pool(name="ob", bufs=4) as pool_ob,        # final out staging
            tc.tile_pool(name="psA", bufs=4, space="PSUM") as pool_psA,  # 1-bank tiles
            tc.tile_pool(name="psS", bufs=2, space="PSUM") as pool_psS,  # scores 2-bank
        ):
            # ---- weight loads ----
            w_tiles = {}
            for name, dram in (("q", wqT), ("k", wkT), ("v", wvT)):
                for kt in range(KT):
                    t = pool_w.tile([P, CL], bf16, tag="w", name=f"w_{name}{kt}")
                    nc.sync.dma_start(out=t, in_=dram[kt * P:(kt + 1) * P, :])
                    w_tiles[(name, kt)] = t
            wp_tiles = []
            for kt in range(MT):
                t = pool_wp.tile([P, C], bf16, tag="wp", name=f"wp{kt}")
                nc.sync.dma_start(out=t, in_=wpT[kt * P:(kt + 1) * P, :])
                wp_tiles.append(t)

            # ---- padded destination tiles (zero halves written once) ----
            # kpad[hl]: [128, N], rows po..po+64 hold k^T_hl, other rows zero.
            kpad = [
                pool_kp.tile([P, N], bf16, tag="kp", name=f"kpad{hl}")
                for hl in range(HPC)
            ]
            for hl in range(HPC):
                zo = (1 - hl % 2) * D
                nc.vector.memset(kpad[hl][zo:zo + D, :], 0.0)
            # vpad[parity][ni]: [128, CL]; for pair hp, cols hp*128+par*64..+64
            # hold v_{2hp+par}, the other 64 cols of the pair block are zero.
            vpad = [
                [
                    pool_v.tile([P, CL], bf16, tag="v", name=f"vpad{par}_{ni}")
                    for ni in range(NT)
                ]
                for par in range(2)
            ]
            for par in range(2):
                for ni in range(NT):
                    for hp in range(MT):
                        zo = hp * P + (1 - par) * D
                        nc.vector.memset(vpad[par][ni][:, zo:zo + D], 0.0)

            # ---- projections (x loads interleaved per tensor to bound SBUF) ----
            qT_tiles = []   # [128, N] x MT  (c_out_local on partitions)

            for name, dram in (("q", xqT), ("k", xkT)):
                x_tiles = []
                for kt in range(KT):
                    t = pool_big.tile([P, N], bf16, tag="big", name=f"x_{name}{kt}")
                    nc.sync.dma_start(out=t, in_=dram[kt * P:(kt + 1) * P, :])
                    x_tiles.append(t)
                for mi in range(MT):
                    if name == "q":
                        out_t = pool_qt.tile([P, N], bf16, tag="qt", name=f"qT{mi}")
                        qT_tiles.append(out_t)
                    for nb in range(NB):
                        ps = pool_psA.tile([P, 512], f32, tag="ps", name="proj_ps")
                        for kt in range(KT):
                            nc.tensor.matmul(
                                ps,
                                lhsT=w_tiles[(name, kt)][:, mi * P:(mi + 1) * P],
                                rhs=x_tiles[kt][:, nb * 512:(nb + 1) * 512],
                                start=(kt == 0),
                                stop=(kt == KT - 1),
                            )
                        if name == "q":
                            nc.vector.tensor_copy(
                                out=out_t[:, nb * 512:(nb + 1) * 512], in_=ps
                            )
                        else:
                            # scatter the head pair into the two padded kT tiles
                            for sub in range(2):
                                hl = mi * 2 + sub
                                po = sub * D
                                nc.vector.tensor_copy(
                                    out=kpad[hl][po:po + D, nb * 512:(nb + 1) * 512],
                                    in_=ps[po:po + D, :],
                                )

            xv_tiles = []
            for kt in range(KT):
                t = pool_big.tile([P, N], bf16, tag="big", name=f"x_v{kt}")
                nc.sync.dma_start(out=t, in_=xvT[kt * P:(kt + 1) * P, :])
                xv_tiles.append(t)
            for ni in range(NT):
                ps = pool_psA.tile([P, 512], f32, tag="ps", name="vproj_ps")
                for kt in range(KT):
                    nc.tensor.matmul(
                        ps,
                        lhsT=xv_tiles[kt][:, ni * P:(ni + 1) * P],
                        rhs=w_tiles[("v", kt)],
                        start=(kt == 0),
                        stop=(kt == KT - 1),
                    )
                # scatter per head parity into masked v tiles
                for par in range(2):
                    for hp in range(MT):
                        co = hp * P + par * D
                        nc.vector.tensor_copy(
                            out=vpad[par][ni][:, co:co + D], in_=ps[:, co:co + D]
                        )

            # ---- attention, head pairs ----
            outz_tiles = [
                pool_oz.tile([P, N], bf16, tag="oz", name=f"outz_{mi}")
                for mi in range(MT)
            ]

            for hp in range(HPC // 2):
                avps = [
                    pool_psA.tile([P, 512], f32, tag="ps", name=f"avps_{hp}_{qb}")
                    for qb in range(NB)
                ]
                for ni in range(NT):
                    strips = [
                        pool_big.tile([P, N], bf16, tag="big", name=f"strip{s}")
                        for s in range(2)
                    ]
                    for qh in range(2):  # two 1024-wide sigmoid chunks
                        spss = [
                            pool_psS.tile([P, 1024], f32, tag="sps", name=f"sps{s}")
                            for s in range(2)
                        ]
                        for qq in range(2):
                            qb = qh * 2 + qq
                            for sub in range(2):
                                nc.tensor.matmul(
                                    spss[sub][:, qq * 512:(qq + 1) * 512],
                                    lhsT=kpad[hp * 2 + sub][:, ni * P:(ni + 1) * P],
                                    rhs=qT_tiles[hp][:, qb * 512:(qb + 1) * 512],
                                    start=True,
                                    stop=True,
                                )
                        for sub in range(2):
                            nc.scalar.activation(
                                strips[sub][:, qh * 1024:(qh + 1) * 1024], spss[sub],
                                SIG, scale=SCALE,
                            )
                    for sub in range(2):
                        nc.sync.dma_start(
                            out=attn_out[hp * 2 + sub, ni * P:(ni + 1) * P, :],
                            in_=strips[sub],
                        )
                    for qb in range(NB):
                        for sub in range(2):
                            nc.tensor.matmul(
                                avps[qb],
                                lhsT=vpad[sub][ni][:, hp * P:(hp + 1) * P],
                                rhs=strips[sub][:, qb * 512:(qb + 1) * 512],
                                start=(ni == 0 and sub == 0),
                                stop=(ni == NT - 1 and sub == 1),
                            )
                for qb in range(NB):
                    nc.vector.tensor_copy(
                        out=outz_tiles[hp][:, qb * 512:(qb + 1) * 512],
                        in_=avps[qb],
                    )

            # ---- final projection: out_part[nq, C] = outz^T.T @ wpT ----
            for mo in range(NT):
                ob = pool_ob.tile([P, C], bf16, tag="ob", name="ob")
                for cb in range(2):
                    ps = pool_psA.tile([P, 512], f32, tag="ps", name="fproj_ps")
                    for kt in range(MT):
                        nc.tensor.matmul(
                            ps,
                            lhsT=outz_tiles[kt][:, mo * P:(mo + 1) * P],
                            rhs=wp_tiles[kt][:, cb * 512:(cb + 1) * 512],
                            start=(kt == 0),
                            stop=(kt == MT - 1),
                        )
                    nc.vector.tensor_copy(out=ob[:, cb * 512:(cb + 1) * 512], in_=ps)
                nc.sync.dma_start(out=out_part[mo * P:(mo + 1) * P, :], in_=ob)

    nc.compile()
    return nc


def _get_nc():
    global _CACHED_NC
    if _CACHED_NC is None:
        _CACHED_NC = _build()
    return _CACHED_NC


def _make_in_maps(inputs):
    in_maps = []
    for i in range(NCORES):
        b, g = i // 2, i % 2
        lo, hi = g * CL, (g + 1) * CL
        in_maps.append({
            "xqT": np.asarray(inputs["x_q"][b]).T.astype(_BF16),
            "xkT": np.asarray(inputs["x_k"][b]).T.astype(_BF16),
            "xvT": np.asarray(inputs["x_v"][b]).T.astype(_BF16),
            "wqT": np.asarray(inputs["Wq"])[lo:hi, :].T.astype(_BF16),
            "wkT": np.asarray(inputs["Wk"])[lo:hi, :].T.astype(_BF16),
            "wvT": np.asarray(inputs["Wv"])[lo:hi, :].T.astype(_BF16),
            "wpT": np.asarray(inputs["Wp"])[:, lo:hi].T.astype(_BF16),
        })
    return in_maps


def _assemble(results, inputs):
    out = np.zeros((B, N, C), np.float32)
    attn = np.empty((B, H, N, N), np.float32)
    for i in range(NCORES):
        b, g = i // 2, i % 2
        r = results[i]
        attn[b, g * HPC:(g + 1) * HPC] = (
            np.asarray(r["attn_out"]).astype(np.float32).transpose(0, 2, 1)
        )
        out[b] += np.asarray(r["out_part"]).astype(np.float32)
    out += np.asarray(inputs["bp"]).astype(np.float32)[None, None, :]
    return out, attn


def run(inputs, trace=False, **kwargs):
    from concourse.bass_utils import run_bass_kernel_spmd

    nc = _get_nc()
    in_maps = _make_in_maps(inputs)
    res = run_bass_kernel_spmd(
        nc, in_maps, core_ids=list(range(NCORES)), trace=trace, **kwargs
    )
    out, attn = _assemble(res.results, inputs)
    return (out, attn), res


def kernel(**inputs):
    (out, attn), _ = run(inputs)
    return out, attn


# revision 13
# speedup vs baseline: 1.0413x; 1.0148x over previous
"""Trainium2 Bass kernel for sigmoid multi-head attention (B=4, N=2048, C=1024, H=16).

Strategy: 8 cores = 4 batches x 2 head-groups (8 heads each). Each core is fully
independent (no collectives):
  - Host pre-transposes + pre-casts inputs to bf16: x^T [C,N], W^T slices.
  - Device computes q^T,k^T (transposed) and v (natural) projections, then per head:
    scores^T[nk,nq] = k^T_h.T @ q^T_h  -> sigmoid (scaled) -> attn^T bf16
    (written to DRAM in [h, nk, nq] layout; host un-transposes),
    out^T_h[d,nq] accumulated as v_h.T @ attn^T, then partial projection
    out_part[nq,C] = outz^T.T @ Wp^T-slice.
  - Host: out[b] = part(core0) + part(core1) + bp; attn un-transposed per head.

All attention matmuls are zero-padded to full 128x128 PE tiles: k^T is stored
per head with the other head's partition rows zeroed, and v is stored per head
parity with the other parity's columns zeroed, so head-pair outputs stack /
accumulate through the zeros. Full-tile matmuls keep LDWEIGHTS on the
background buffer path (~225 ns/MM) instead of serializing (~270-330 ns/MM).
"""

import numpy as np
import ml_dtypes

B, N, C, H = 4, 2048, 1024, 16
D = C // H            # 64
HPC = H // 2          # 8 heads per core
CL = HPC * D          # 512 local channels
NCORES = 8
SCALE = D ** -0.5

P = 128
KT = C // P           # 8  k-tiles over c_in
MT = CL // P          # 4  tiles over local channels
NT = N // P           # 16 tiles over sequence
NB = N // 512         # 4  512-wide banks over sequence

_BF16 = ml_dtypes.bfloat16

_CACHED_NC = None


def _build():
    import concourse.mybir as mybir
    import concourse.tile as tile
    from concourse import bacc

    bf16 = mybir.dt.bfloat16
    f32 = mybir.dt.float32
    SIG = mybir.ActivationFunctionType.Sigmoid

    nc = bacc.Bacc("TRN2")

    xqT = nc.declare_dram_parameter("xqT", [C, N], bf16, isOutput=False)
    xkT = nc.declare_dram_parameter("xkT", [C, N], bf16, isOutput=False)
    xvT = nc.declare_dram_parameter("xvT", [C, N], bf16, isOutput=False)
    wqT = nc.declare_dram_parameter("wqT", [C, CL], bf16, isOutput=False)
    wkT = nc.declare_dram_parameter("wkT", [C, CL], bf16, isOutput=False)
    wvT = nc.declare_dram_parameter("wvT", [C, CL], bf16, isOutput=False)
    wpT = nc.declare_dram_parameter("wpT", [CL, C], bf16, isOutput=False)
    attn_out = nc.declare_dram_parameter("attn_out", [HPC, N, N], bf16, isOutput=True)
    out_part = nc.declare_dram_parameter("out_part", [N, C], bf16, isOutput=True)

    with tile.TileContext(nc) as tc:
        with (
            tc.tile_pool(name="big", bufs=16) as pool_big,     # x k-tiles + attnT strips
            tc.tile_pool(name="qt", bufs=MT) as pool_qt,       # qT tiles, live all run
            tc.tile_pool(name="kp", bufs=HPC) as pool_kp,      # padded kT per head
            tc.tile_pool(name="vp", bufs=2 * NT) as pool_v,    # padded v per parity
            tc.tile_pool(name="w", bufs=2 * KT) as pool_w,     # w k-tiles (2 phases live)
            tc.tile_pool(name="wp", bufs=MT) as pool_wp,
            tc.tile_pool(name="oz", bufs=MT) as pool_oz,       # outz^T bf16
            tc.tile_pool(name="ob", bufs=4) as pool_ob,        # final out staging
            tc.tile_pool(name="psA", bufs=4, space="PSUM") as pool_psA,  # 1-bank tiles
            tc.tile_pool(name="psS", bufs=2, space="PSUM") as pool_psS,  # scores 2-bank
        ):
            # ---- padded destination tiles (zero halves written once) ----
            # kpad[hl]: [128, N], rows po..po+64 hold k^T_hl, other rows zero.
            kpad = [
                pool_kp.tile([P, N], bf16, tag="kp", name=f"kpad{hl}")
                for hl in range(HPC)
            ]
            for hl in range(HPC):
                zo = (1 - hl % 2) * D
                nc.vector.memset(kpad[hl][zo:zo + D, :], 0.0)
            # vpad[parity][ni]: [128, CL]; for pair hp, cols hp*128+par*64..+64
            # hold v_{2hp+par}, the other 64 cols of the pair block are zero.
            vpad = [
                [
                    pool_v.tile([P, CL], bf16, tag="v", name=f"vpad{par}_{ni}")
                    for ni in range(NT)
                ]
                for par in range(2)
            ]
            for par in range(2):
                for ni in range(NT):
                    for hp in range(MT):
                        zo = hp * P + (1 - par) * D
                        nc.vector.memset(vpad[par][ni][:, zo:zo + D], 0.0)

            # ---- projections: per-tensor phases (weights + x loads + matmuls)
            # so the q/k phases start DMA+PE as early as possible and the
            # attention pipeline can begin while the v phase still loads.
            w_tiles = {}
            qT_tiles = []   # [128, N] x MT  (c_out_local on partitions)

            for name, wdram, dram in (("q", wqT, xqT), ("k", wkT, xkT)):
                x_tiles = []
                for kt in range(KT):
                    wt = pool_w.tile([P, CL], bf16, tag="w", name=f"w_{name}{kt}")
                    nc.sync.dma_start(out=wt, in_=wdram[kt * P:(kt + 1) * P, :])
                    w_tiles[(name, kt)] = wt
                    t = pool_big.tile([P, N], bf16, tag="big", name=f"x_{name}{kt}")
                    nc.sync.dma_start(out=t, in_=dram[kt * P:(kt + 1) * P, :])
                    x_tiles.append(t)
                for mi in range(MT):
                    if name == "q":
                        out_t = pool_qt.tile([P, N], bf16, tag="qt", name=f"qT{mi}")
                        qT_tiles.append(out_t)
                    for nb in range(NB):
                        ps = pool_psA.tile([P, 512], f32, tag="ps", name="proj_ps")
                        for kt in range(KT):
                            nc.tensor.matmul(
                                ps,
                                lhsT=w_tiles[(name, kt)][:, mi * P:(mi + 1) * P],
                                rhs=x_tiles[kt][:, nb * 512:(nb + 1) * 512],
                                start=(kt == 0),
                                stop=(kt == KT - 1),
                            )
                        if name == "q":
                            nc.vector.tensor_copy(
                                out=out_t[:, nb * 512:(nb + 1) * 512], in_=ps
                            )
                        else:
                            # scatter the head pair into the two padded kT tiles
                            for sub in range(2):
                                hl = mi * 2 + sub
                                po = sub * D
                                nc.vector.tensor_copy(
                                    out=kpad[hl][po:po + D, nb * 512:(nb + 1) * 512],
                                    in_=ps[po:po + D, :],
                                )

            xv_tiles = []
            for kt in range(KT):
                wt = pool_w.tile([P, CL], bf16, tag="w", name=f"w_v{kt}")
                nc.sync.dma_start(out=wt, in_=wvT[kt * P:(kt + 1) * P, :])
                w_tiles[("v", kt)] = wt
                t = pool_big.tile([P, N], bf16, tag="big", name=f"x_v{kt}")
                nc.sync.dma_start(out=t, in_=xvT[kt * P:(kt + 1) * P, :])
                xv_tiles.append(t)
            wp_tiles = []
            for kt in range(MT):
                wt = pool_wp.tile([P, C], bf16, tag="wp", name=f"wp{kt}")
                nc.sync.dma_start(out=wt, in_=wpT[kt * P:(kt + 1) * P, :])
                wp_tiles.append(wt)
            for ni in range(NT):
                ps = pool_psA.tile([P, 512], f32, tag="ps", name="vproj_ps")
                for kt in range(KT):
                    nc.tensor.matmul(
                        ps,
                        lhsT=xv_tiles[kt][:, ni * P:(ni + 1) * P],
                        rhs=w_tiles[("v", kt)],
                        start=(kt == 0),
                        stop=(kt == KT - 1),
                    )
                # scatter per head parity into masked v tiles
                for par in range(2):
                    for hp in range(MT):
                        co = hp * P + par * D
                        nc.vector.tensor_copy(
                            out=vpad[par][ni][:, co:co + D], in_=ps[:, co:co + D]
                        )

            # ---- attention, head pairs ----
            outz_tiles = [
                pool_oz.tile([P, N], bf16, tag="oz", name=f"outz_{mi}")
                for mi in range(MT)
            ]

            for hp in range(HPC // 2):
                avps = [
                    pool_psA.tile([P, 512], f32, tag="ps", name=f"avps_{hp}_{qb}")
                    for qb in range(NB)
                ]
                for ni in range(NT):
                    strips = [
                        pool_big.tile([P, N], bf16, tag="big", name=f"strip{s}")
                        for s in range(2)
                    ]
                    for qh in range(2):  # two 1024-wide sigmoid chunks
                        spss = [
                            pool_psS.tile([P, 1024], f32, tag="sps", name=f"sps{s}")
                            for s in range(2)
                        ]
                        for qq in range(2):
                            qb = qh * 2 + qq
                            for sub in range(2):
                                nc.tensor.matmul(
                                    spss[sub][:, qq * 512:(qq + 1) * 512],
                                    lhsT=kpad[hp * 2 + sub][:, ni * P:(ni + 1) * P],
                                    rhs=qT_tiles[hp][:, qb * 512:(qb + 1) * 512],
                                    start=True,
                                    stop=True,
                                )
                        for sub in range(2):
                            nc.scalar.activation(
                                strips[sub][:, qh * 1024:(qh + 1) * 1024], spss[sub],
                                SIG, scale=SCALE,
                            )
                    for sub in range(2):
                        nc.sync.dma_start(
                            out=attn_out[hp * 2 + sub, ni * P:(ni + 1) * P, :],
                            in_=strips[sub],
                        )
                    for qb in range(NB):
                        for sub in range(2):
                            nc.tensor.matmul(
                                avps[qb],
                                lhsT=vpad[sub][ni][:, hp * P:(hp + 1) * P],
                                rhs=strips[sub][:, qb * 512:(qb + 1) * 512],
                                start=(ni == 0 and sub == 0),
                                stop=(ni == NT - 1 and sub == 1),
                            )
                for qb in range(NB):
                    nc.vector.tensor_copy(
                        out=outz_tiles[hp][:, qb * 512:(qb + 1) * 512],
                        in_=avps[qb],
                    )

            # ---- final projection: out_part[nq, C] = outz^T.T @ wpT ----
            for mo in range(NT):
                ob = pool_ob.tile([P, C], bf16, tag="ob", name="ob")
                for cb in range(2):
                    ps = pool_psA.tile([P, 512], f32, tag="ps", name="fproj_ps")
                    for kt in range(MT):
                        nc.tensor.matmul(
                            ps,
                            lhsT=outz_tiles[kt][:, mo * P:(mo + 1) * P],
                            rhs=wp_tiles[kt][:, cb * 512:(cb + 1) * 512],
                            start=(kt == 0),
                            stop=(kt == MT - 1),
                        )
                    nc.vector.tensor_copy(out=ob[:, cb * 512:(cb + 1) * 512], in_=ps)
                nc.sync.dma_start(out=out_part[mo * P:(mo + 1) * P, :], in_=ob)

    nc.compile()
    return nc


def _get_nc():
    global _CACHED_NC
    if _CACHED_NC is None:
        _CACHED_NC = _build()
    return _CACHED_NC


def _make_in_maps(inputs):
    in_maps = []
    for i in range(NCORES):
        b, g = i // 2, i % 2
        lo, hi = g * CL, (g + 1) * CL
        in_maps.append({
            "xqT": np.asarray(inputs["x_q"][b]).T.astype(_BF16),
            "xkT": np.asarray(inputs["x_k"][b]).T.astype(_BF16),
            "xvT": np.asarray(inputs["x_v"][b]).T.astype(_BF16),
            "wqT": np.asarray(inputs["Wq"])[lo:hi, :].T.astype(_BF16),
            "wkT": np.asarray(inputs["Wk"])[lo:hi, :].T.astype(_BF16),
            "wvT": np.asarray(inputs["Wv"])[lo:hi, :].T.astype(_BF16),
            "wpT": np.asarray(inputs["Wp"])[:, lo:hi].T.astype(_BF16),
        })
    return in_maps


def _assemble(results, inputs):
    out = np.zeros((B, N, C), np.float32)
    attn = np.empty((B, H, N, N), np.float32)
    for i in range(NCORES):
        b, g = i // 2, i % 2
        r = results[i]
        attn[b, g * HPC:(g + 1) * HPC] = (
            np.asarray(r["attn_out"]).astype(np.float32).transpose(0, 2, 1)
        )
        out[b] += np.asarray(r["out_part"]).astype(np.float32)
    out += np.asarray(inputs["bp"]).astype(np.float32)[None, None, :]
    return out, attn


def run(inputs, trace=False, **kwargs):
    from concourse.bass_utils import run_bass_kernel_spmd

    nc = _get_nc()
    in_maps = _make_in_maps(inputs)
    res = run_bass_kernel_spmd(
        nc, in_maps, core_ids=list(range(NCORES)), trace=trace, **kwargs
    )
    out, attn = _assemble(res.results, inputs)
    return (out, attn), res


def kernel(**inputs):
    (out, attn), _ = run(inputs)
    return out, attn
